# revision 17
# baseline (speedup 1.0000x reference)
"""Trainium2 Bass kernel for nn_ContrastiveLoss (NT-Xent / SimCLR loss).

B=4096, D=512, 100 classes, temperature 0.5.  loss =
  mean_i [ log(denom_i + 1e-7) - pos_i/t ]
where denom_i = sum_{j: label_j != label_i} exp(sim_ij/t) + exp(pos_i/t).

Distribution: rows of the similarity matrix are sharded across 8 cores
(1024 rows each).  Host passes bf16 inputs ROTATED per core so that rows
0..1023 of x_rot are the core's own rows — the core then builds the
transposed, normalized, fp8 representation matrix ZT column-group by
column-group starting with its own rows, so the first matmul wave starts
~15 us in and the PE stays continuously busy (keeps the HAM clock warm).

Compute plan (per core):
  PE   - fp8e4 DoubleRow matmuls: z16 = 16*x/||x|| in fp8, K=512 done as
         2 pair-chunks of (2x128); PSUM gets 256*sim.  Label mask folded
         in as a bf16 one-hot matmul: -64*onehot x 64*onehot accumulates
         -4096 into same-label entries; exp((256*sim-4096)/128) = 0.
  ACT  - ONLY Exp over [128, 2048] PSUM tiles (fused accum_out = row
         denominator) + two batched Sqrt instrs + final Ln: 5 activation
         table loads total instead of 16.
  DVE  - fused square+reduce (tensor_tensor_reduce) for row norms,
         reciprocal, z16 scaling, bf16->fp8 convert of transposed tiles,
         one-hot build.
  DMA  - bf16 input loads (half the f32 traffic) + xbar transposes.
ZT is stored k-major [kappa][k_chunk][block][rho] so DoubleRow rhs slices
optimize to legal 3D APs with 512-wide moving operands.
Positives are recomputed exactly from bf16 inputs (praw * 16/ni * 16/nj
/ 128) and re-added to the denominator.
Final reduction: ones-vector matmul -> [1,1]; host sums 8 partials.
"""

import sys

for _p in ("/opt/trn_rl_repo", "/root/.axon_site/_ro/trn_rl_repo"):
    if _p not in sys.path:
        sys.path.append(_p)

import numpy as np
import ml_dtypes

import concourse.bass as bass
import concourse.bacc as bacc
import concourse.mybir as mybir
from concourse import tile
from concourse.bass_utils import run_bass_kernel_spmd

F32 = mybir.dt.float32
BF16 = mybir.dt.bfloat16
FP8 = mybir.dt.float8e4
AF = mybir.ActivationFunctionType
ALU = mybir.AluOpType
AX = mybir.AxisListType
DR = mybir.MatmulPerfMode.DoubleRow

P = 128          # partitions
B = 4096         # batch
D = 512          # embedding dim
N2 = 2 * B       # 8192 rows of sim matrix
NCORES = 8
MYR = N2 // NCORES          # 1024 rows per core
NB = N2 // P                # 64 row blocks total
MB = MYR // P               # 8 row blocks per core
KD = D // P                 # 4 contraction chunks
GB = 8                      # row blocks per load group
NG = 8                      # load groups
COLS = 1024                 # psum tile columns (one load group per wave)
NW = N2 // COLS             # 8 column waves
ZSC = 16.0                  # fp8 scale: z16 = 16 * z
EXPS = 2.0 / (ZSC * ZSC)    # exp scale: 2/256 = 1/128 (temp 0.5)
MASK_W = 64.0               # -64*64 = -4096 -> exp shift -32
USE_DR = True               # DoubleRow fp8 matmuls (2 k-chunks / instr)


def build_program():
    nc = bacc.Bacc("TRN2", target_bir_lowering=False, debug=False)

    x_rot = nc.dram_tensor("x_rot", [N2, D], BF16, kind="ExternalInput").ap()
    pt_x = nc.dram_tensor("pt_x", [MYR, D], BF16, kind="ExternalInput").ap()
    labels_rot = nc.dram_tensor("labels_rot", [1, N2], BF16, kind="ExternalInput").ap()
    iota_p = nc.dram_tensor("iota_p", [P, 1], F32, kind="ExternalInput").ap()
    ones_p = nc.dram_tensor("ones_p", [P, 1], F32, kind="ExternalInput").ap()
    out_loss = nc.dram_tensor("out_loss", [1, 1], F32, kind="ExternalOutput").ap()

    with tile.TileContext(nc) as tc:
        with (
            tc.tile_pool(name="big", bufs=1) as big,
            tc.tile_pool(name="xin", bufs=1) as xin,
            tc.tile_pool(name="zs", bufs=2) as zs,
            tc.tile_pool(name="ztb", bufs=2) as ztb,
            tc.tile_pool(name="ebuf", bufs=3) as ebuf,
            tc.tile_pool(name="small", bufs=1) as small,
            tc.tile_pool(name="pmm", bufs=4, space=bass.MemorySpace.PSUM) as pmm,
        ):
            # ---- persistent tiles ----
            ZT = big.tile([P, KD, NB, P], FP8, name="ZT")      # [kappa][k][blk][rho]
            LTb = big.tile([P, N2], BF16, name="LTb")          # 64*onehot (rhs)
            LTa = big.tile([P, MYR], BF16, name="LTa")         # -64*onehot (lhsT)

            S = small.tile([P, NB], F32, name="S")             # ||x||^2 per row
            SPT = small.tile([P, MB], F32, name="SPT")
            NRM = small.tile([P, NB], F32, name="NRM")
            NRMPT = small.tile([P, MB], F32, name="NRMPT")
            RS = small.tile([P, NB], F32, name="RS")           # nrm/16
            RSPT = small.tile([P, MB], F32, name="RSPT")
            R16 = small.tile([P, NB], F32, name="R16")         # 16/nrm
            R16PT = small.tile([P, MB], F32, name="R16PT")
            SCR = small.tile([P, D], BF16, name="SCR")         # DVE praw scratch
            SQ = small.tile([P, D], BF16, name="SQ")           # ACT square scratch
            Praw = small.tile([P, MB], F32, name="Praw")
            P2 = small.tile([P, MB], F32, name="P2")           # positives / t
            ACC = small.tile([P, MB, NW], F32, name="ACC")
            DSUM = small.tile([P, MB], F32, name="DSUM")
            NOM = small.tile([P, MB], F32, name="NOM")
            DEN = small.tile([P, MB], F32, name="DEN")
            LOSS = small.tile([P, MB], F32, name="LOSS")
            LOSS2 = small.tile([P, MB], F32, name="LOSS2")
            TOT = small.tile([P, 1], F32, name="TOT")
            IOT = small.tile([P, 1], F32, name="IOT")
            ONE = small.tile([P, 1], F32, name="ONE")
            EPS = small.tile([P, 1], F32, name="EPS")
            nc.vector.memset(EPS[:], 1e-7)

            nc.sync.dma_start(out=IOT[:], in_=iota_p)
            nc.sync.dma_start(out=ONE[:], in_=ones_p)

            # ---- input loads (bf16); loads g0,g1 first, then labels, pt ----
            xg = []
            for g in range(NG):
                t = xin.tile([P, GB, D], BF16, name=f"xg{g}", tag="xg", bufs=NG)
                xg.append(t)
            pxg = xin.tile([P, MB, D], BF16, name="pxg", tag="px", bufs=1)

            def load_group(g):
                src = x_rot[g * GB * P:(g + 1) * GB * P, :].rearrange(
                    "(b p) d -> p b d", p=P)
                nc.sync.dma_start(out=xg[g][:], in_=src)

            load_group(0)
            load_group(1)
            nc.sync.dma_start(out=pxg[:], in_=pt_x.rearrange("(b p) d -> p b d", p=P))
            nc.sync.dma_start(out=LTb[:], in_=labels_rot.partition_broadcast(P))
            for g in range(2, NG):
                load_group(g)

            # ---- ACT: squares (Square + fused accum), sqrt batches ----
            def squares_group(g):
                for j in range(GB):
                    b = g * GB + j
                    nc.scalar.activation(
                        SQ[:], xg[g][:, j, :], AF.Square,
                        accum_out=S[:, b:b + 1])

            squares_group(0)
            squares_group(1)
            nc.scalar.activation(NRM[:, 0:2 * GB], S[:, 0:2 * GB], AF.Sqrt)
            for j in range(MB):
                nc.scalar.activation(
                    SQ[:], pxg[:, j, :], AF.Square, accum_out=SPT[:, j:j + 1])
            for g in range(2, NG):
                squares_group(g)
            nc.scalar.activation(NRM[:, 2 * GB:], S[:, 2 * GB:], AF.Sqrt)
            nc.scalar.activation(NRMPT[:], SPT[:], AF.Sqrt)

            # ---- DVE: reciprocals, scales; GpSimd: fp8 converts ----
            def recip_range(lo, hi):
                nc.vector.tensor_scalar(
                    out=RS[:, lo:hi], in0=NRM[:, lo:hi],
                    scalar1=1.0 / ZSC, scalar2=None, op0=ALU.mult)
                nc.vector.reciprocal(R16[:, lo:hi], RS[:, lo:hi])

            def prep_group(g):
                # z16 = x * (16/||x||)  (bf16) -> transpose -> fp8 convert
                zg = zs.tile([P, GB, D], BF16, name=f"zg{g}", tag="zg")
                for j in range(GB):
                    b = g * GB + j
                    nc.vector.tensor_scalar(
                        out=zg[:, j, :], in0=xg[g][:, j, :],
                        scalar1=R16[:, b:b + 1], scalar2=None, op0=ALU.mult)
                zt = ztb.tile([P, GB, KD, P], BF16, name=f"zt{g}", tag="zt")
                nc.scalar.dma_start_transpose(out=zt[:], in_=zg[:])
                for k in range(KD):
                    nc.gpsimd.tensor_scalar(
                        out=ZT[:, k, g * GB:(g + 1) * GB, :],
                        in0=zt[:, :, k, :],
                        scalar1=1.0, scalar2=None, op0=ALU.mult)

            recip_range(0, 2 * GB)
            prep_group(0)
            prep_group(1)

            # one-hot masks (DVE, after scales so they don't delay wave 0)
            nc.vector.tensor_scalar(
                out=LTa[:], in0=LTb[:, 0:MYR], scalar1=IOT[:], scalar2=-MASK_W,
                op0=ALU.is_equal, op1=ALU.mult,
            )
            nc.vector.tensor_scalar(
                out=LTb[:], in0=LTb[:], scalar1=IOT[:], scalar2=MASK_W,
                op0=ALU.is_equal, op1=ALU.mult,
            )

            # positives (DVE; needed only for the epilogue)
            def positives():
                for j in range(MB):
                    nc.vector.tensor_mul(SCR[:], xg[0][:, j, :], pxg[:, j, :])
                    nc.vector.tensor_reduce(
                        Praw[:, j:j + 1], SCR[:], axis=AX.X, op=ALU.add)
                nc.vector.tensor_scalar(
                    out=RSPT[:], in0=NRMPT[:], scalar1=1.0 / ZSC, scalar2=None,
                    op0=ALU.mult)
                nc.vector.reciprocal(R16PT[:], RSPT[:])
                nc.vector.tensor_mul(P2[:], Praw[:], R16[:, 0:MB])
                nc.vector.tensor_mul(P2[:], P2[:], R16PT[:])
                nc.vector.tensor_scalar(
                    out=P2[:], in0=P2[:], scalar1=EXPS, scalar2=None,
                    op0=ALU.mult)

            # ---- main loop: 8 column waves x 8 row blocks ----
            def mm_tile(ngi, m):
                ps = pmm.tile([P, COLS], F32, name=f"ps{ngi}_{m}", tag="mm")
                if USE_DR:
                    for kp in range(0, KD, 2):
                        lhsT = ZT[:, kp:kp + 2, m, :]
                        for ns in range(COLS // 512):
                            b0 = ngi * (COLS // P) + ns * 4
                            rhs = ZT[:, kp:kp + 2, b0:b0 + 4, :]
                            nc.tensor.matmul(
                                ps[:, ns * 512:(ns + 1) * 512], lhsT, rhs,
                                start=(kp == 0), stop=False, perf_mode=DR)
                else:
                    for k in range(KD):
                        lhsT = ZT[:, k, m, :]
                        for ns in range(COLS // 512):
                            b0 = ngi * (COLS // P) + ns * 4
                            rhs = ZT[:, k, b0:b0 + 4, :]
                            nc.tensor.matmul(
                                ps[:, ns * 512:(ns + 1) * 512], lhsT, rhs,
                                start=(k == 0), stop=False)
                for ns in range(COLS // 512):
                    c0 = ngi * COLS + ns * 512
                    nc.tensor.matmul(
                        ps[:, ns * 512:(ns + 1) * 512],
                        LTa[:, m * P:(m + 1) * P], LTb[:, c0:c0 + 512],
                        start=False, stop=True)
                e = ebuf.tile([P, COLS], BF16, name=f"e{ngi}_{m}", tag="e")
                nc.scalar.activation(
                    e[:], ps[:], AF.Exp, scale=EXPS,
                    accum_out=ACC[:, m, ngi:ngi + 1])

            for ngi in range(NW):
                for m in range(MB):
                    mm_tile(ngi, m)
                    if ngi == 0 and m == 5:
                        recip_range(2 * GB, NB)
                        positives()
                    if ngi == 0 and m == 7:
                        nc.scalar.activation(NOM[:], P2[:], AF.Exp)
                    if m == 3 and 1 <= ngi <= 6:
                        prep_group(ngi + 1)

            # ---- batched epilogue ----
            nc.vector.tensor_reduce(DSUM[:], ACC[:], axis=AX.X, op=ALU.add)
            nc.vector.tensor_add(DEN[:], DSUM[:], NOM[:])
            nc.scalar.activation(LOSS[:], DEN[:], AF.Ln, bias=EPS[:])
            nc.vector.tensor_sub(LOSS2[:], LOSS[:], P2[:])
            nc.vector.tensor_reduce(TOT[:], LOSS2[:], axis=AX.X, op=ALU.add)
            psc = pmm.tile([1, 1], F32, name="psc", tag="mm")
            nc.tensor.matmul(psc[:], TOT[:], ONE[:], start=True, stop=True)
            osb = small.tile([1, 1], F32, name="osb")
            nc.scalar.copy(osb[:], psc[:])
            nc.sync.dma_start(out=out_loss, in_=osb[:])

    nc.compile()
    return nc


_NC_CACHE = None
LAST_RESULTS = None  # test harness can read exec_time_ns / trace from here


def _get_nc():
    global _NC_CACHE
    if _NC_CACHE is None:
        _NC_CACHE = build_program()
    return _NC_CACHE


def kernel(emb_i, emb_j, target):
    emb_i = np.ascontiguousarray(emb_i, dtype=np.float32)
    emb_j = np.ascontiguousarray(emb_j, dtype=np.float32)
    target = np.asarray(target)

    X = np.concatenate([emb_i, emb_j], axis=0).astype(ml_dtypes.bfloat16)
    labels = np.concatenate([target, target]).astype(np.float32)
    labels_bf = labels.astype(ml_dtypes.bfloat16)

    iota_p = np.arange(P, dtype=np.float32).reshape(P, 1)
    ones_p = np.ones((P, 1), dtype=np.float32)

    in_maps = []
    for c in range(NCORES):
        lo = c * MYR
        x_rot = np.ascontiguousarray(np.concatenate([X[lo:], X[:lo]], axis=0))
        lab_rot = np.ascontiguousarray(
            np.concatenate([labels_bf[lo:], labels_bf[:lo]])).reshape(1, N2)
        pt_idx = (np.arange(lo, lo + MYR) + B) % N2
        in_maps.append({
            "x_rot": x_rot,
            "pt_x": np.ascontiguousarray(X[pt_idx]),
            "labels_rot": lab_rot,
            "iota_p": iota_p,
            "ones_p": ones_p,
        })

    nc = _get_nc()
    res = run_bass_kernel_spmd(nc, in_maps, core_ids=list(range(NCORES)))
    global LAST_RESULTS
    LAST_RESULTS = res
    total = 0.0
    for c in range(NCORES):
        total += float(res.results[c]["out_loss"][0, 0])
    return np.float32(total / N2)


# revision 18
# speedup vs baseline: 2.7404x; 2.7404x over previous
"""Trainium2 Bass kernel for nn_ContrastiveLoss (NT-Xent / SimCLR loss).

B=4096, D=512, 100 classes, temperature 0.5.  loss =
  mean_i [ log(denom_i + 1e-7) - pos_i/t ]
where denom_i = sum_{j: label_j != label_i} exp(sim_ij/t) + exp(pos_i/t).

Distribution: rows of the similarity matrix are sharded across 8 cores
(1024 rows each).  Host passes bf16 inputs ROTATED per core so that rows
0..1023 of x_rot are the core's own rows — the core then builds the
transposed, normalized, fp8 representation matrix ZT column-group by
column-group starting with its own rows, so the first matmul wave starts
~15 us in and the PE stays continuously busy (keeps the HAM clock warm).

Compute plan (per core):
  PE   - fp8e4 DoubleRow matmuls: z16 = 16*x/||x|| in fp8, K=512 done as
         2 pair-chunks of (2x128); PSUM gets 256*sim.  Label mask folded
         in as a bf16 one-hot matmul: -64*onehot x 64*onehot accumulates
         -4096 into same-label entries; exp((256*sim-4096)/128) = 0.
  ACT  - ONLY Exp over [128, 2048] PSUM tiles (fused accum_out = row
         denominator) + two batched Sqrt instrs + final Ln: 5 activation
         table loads total instead of 16.
  DVE  - fused square+reduce (tensor_tensor_reduce) for row norms,
         reciprocal, z16 scaling, bf16->fp8 convert of transposed tiles,
         one-hot build.
  DMA  - bf16 input loads (half the f32 traffic) + xbar transposes.
ZT is stored k-major [kappa][k_chunk][block][rho] so DoubleRow rhs slices
optimize to legal 3D APs with 512-wide moving operands.
Positives are recomputed exactly from bf16 inputs (praw * 16/ni * 16/nj
/ 128) and re-added to the denominator.
Final reduction: ones-vector matmul -> [1,1]; host sums 8 partials.
"""

import sys

for _p in ("/opt/trn_rl_repo", "/root/.axon_site/_ro/trn_rl_repo"):
    if _p not in sys.path:
        sys.path.append(_p)

import numpy as np
import ml_dtypes

import concourse.bass as bass
import concourse.bacc as bacc
import concourse.mybir as mybir
from concourse import tile
from concourse.bass_utils import run_bass_kernel_spmd

F32 = mybir.dt.float32
BF16 = mybir.dt.bfloat16
FP8 = mybir.dt.float8e4
AF = mybir.ActivationFunctionType
ALU = mybir.AluOpType
AX = mybir.AxisListType
DR = mybir.MatmulPerfMode.DoubleRow

P = 128          # partitions
B = 4096         # batch
D = 512          # embedding dim
N2 = 2 * B       # 8192 rows of sim matrix
NCORES = 8
MYR = N2 // NCORES          # 1024 rows per core
NB = N2 // P                # 64 row blocks total
MB = MYR // P               # 8 row blocks per core
KD = D // P                 # 4 contraction chunks
GB = 8                      # row blocks per load group
NG = 8                      # load groups
COLS = 2048                 # psum tile columns
NW = N2 // COLS             # 4 column waves
ZSC = 16.0                  # fp8 scale: z16 = 16 * z
EXPS = 2.0 / (ZSC * ZSC)    # exp scale: 2/256 = 1/128 (temp 0.5)
MASK_W = 64.0               # -64*64 = -4096 -> exp shift -32
USE_DR = True               # DoubleRow fp8 matmuls (2 k-chunks / instr)


def build_program():
    nc = bacc.Bacc("TRN2", target_bir_lowering=False, debug=False)

    x_rot = nc.dram_tensor("x_rot", [N2, D], BF16, kind="ExternalInput").ap()
    pt_x = nc.dram_tensor("pt_x", [MYR, D], BF16, kind="ExternalInput").ap()
    labels_rot = nc.dram_tensor("labels_rot", [1, N2], BF16, kind="ExternalInput").ap()
    iota_p = nc.dram_tensor("iota_p", [P, 1], F32, kind="ExternalInput").ap()
    ones_p = nc.dram_tensor("ones_p", [P, 1], F32, kind="ExternalInput").ap()
    out_loss = nc.dram_tensor("out_loss", [1, 1], F32, kind="ExternalOutput").ap()

    with tile.TileContext(nc) as tc:
        with (
            tc.tile_pool(name="big", bufs=1) as big,
            tc.tile_pool(name="xin", bufs=1) as xin,
            tc.tile_pool(name="zs", bufs=2) as zs,
            tc.tile_pool(name="ztb", bufs=2) as ztb,
            tc.tile_pool(name="ebuf", bufs=3) as ebuf,
            tc.tile_pool(name="small", bufs=1) as small,
            tc.tile_pool(name="pmm", bufs=2, space=bass.MemorySpace.PSUM) as pmm,
        ):
            # ---- persistent tiles ----
            ZT = big.tile([P, KD, NB, P], FP8, name="ZT")      # [kappa][k][blk][rho]
            LTb = big.tile([P, N2], BF16, name="LTb")          # 64*onehot (rhs)
            LTa = big.tile([P, MYR], BF16, name="LTa")         # -64*onehot (lhsT)

            S = small.tile([P, NB], F32, name="S")             # ||x||^2 per row
            SPT = small.tile([P, MB], F32, name="SPT")
            NRM = small.tile([P, NB], F32, name="NRM")
            NRMPT = small.tile([P, MB], F32, name="NRMPT")
            RS = small.tile([P, NB], F32, name="RS")           # nrm/16
            RSPT = small.tile([P, MB], F32, name="RSPT")
            R16 = small.tile([P, NB], F32, name="R16")         # 16/nrm
            R16PT = small.tile([P, MB], F32, name="R16PT")
            SCR = small.tile([P, D], BF16, name="SCR")         # ttr scratch
            Praw = small.tile([P, MB], F32, name="Praw")
            P2 = small.tile([P, MB], F32, name="P2")           # positives / t
            ACC = small.tile([P, MB, NW], F32, name="ACC")
            DSUM = small.tile([P, MB], F32, name="DSUM")
            NOM = small.tile([P, MB], F32, name="NOM")
            DEN = small.tile([P, MB], F32, name="DEN")
            LOSS = small.tile([P, MB], F32, name="LOSS")
            LOSS2 = small.tile([P, MB], F32, name="LOSS2")
            TOT = small.tile([P, 1], F32, name="TOT")
            IOT = small.tile([P, 1], F32, name="IOT")
            ONE = small.tile([P, 1], F32, name="ONE")
            EPS = small.tile([P, 1], F32, name="EPS")
            nc.vector.memset(EPS[:], 1e-7)

            nc.sync.dma_start(out=IOT[:], in_=iota_p)
            nc.sync.dma_start(out=ONE[:], in_=ones_p)

            # ---- label one-hot masks (LTb built in place over the bcast) ----
            nc.sync.dma_start(out=LTb[:], in_=labels_rot.partition_broadcast(P))
            nc.vector.tensor_scalar(
                out=LTa[:], in0=LTb[:, 0:MYR], scalar1=IOT[:], scalar2=-MASK_W,
                op0=ALU.is_equal, op1=ALU.mult,
            )
            nc.vector.tensor_scalar(
                out=LTb[:], in0=LTb[:], scalar1=IOT[:], scalar2=MASK_W,
                op0=ALU.is_equal, op1=ALU.mult,
            )

            # ---- input loads (bf16) ----
            xg = []
            for g in range(NG):
                t = xin.tile([P, GB, D], BF16, name=f"xg{g}", tag="xg", bufs=NG)
                xg.append(t)
            pxg = xin.tile([P, MB, D], BF16, name="pxg", tag="px", bufs=1)

            def load_group(g):
                src = x_rot[g * GB * P:(g + 1) * GB * P, :].rearrange(
                    "(b p) d -> p b d", p=P)
                nc.sync.dma_start(out=xg[g][:], in_=src)

            def sq_reduce(in0, in1, acc):
                nc.vector.tensor_mul(SCR[:], in0, in1)
                nc.vector.tensor_reduce(acc, SCR[:], axis=AX.X, op=ALU.add)

            def squares_group(g):
                for j in range(GB):
                    b = g * GB + j
                    sq_reduce(xg[g][:, j, :], xg[g][:, j, :], S[:, b:b + 1])

            def prep_group(g):
                # z16 = x * (16/||x||)  (bf16) -> transpose -> fp8 convert
                zg = zs.tile([P, GB, D], BF16, name=f"zg{g}", tag="zg")
                for j in range(GB):
                    b = g * GB + j
                    nc.vector.tensor_scalar(
                        out=zg[:, j, :], in0=xg[g][:, j, :],
                        scalar1=R16[:, b:b + 1], scalar2=None, op0=ALU.mult)
                zt = ztb.tile([P, GB, KD, P], BF16, name=f"zt{g}", tag="zt")
                nc.scalar.dma_start_transpose(out=zt[:], in_=zg[:])
                for k in range(KD):
                    nc.vector.tensor_scalar(
                        out=ZT[:, k, g * GB:(g + 1) * GB, :],
                        in0=zt[:, :, k, :],
                        scalar1=1.0, scalar2=None, op0=ALU.mult)

            # group 0 (= my rows) + partner first
            load_group(0)
            nc.sync.dma_start(out=pxg[:], in_=pt_x.rearrange("(b p) d -> p b d", p=P))
            squares_group(0)
            for j in range(MB):
                sq_reduce(pxg[:, j, :], pxg[:, j, :], SPT[:, j:j + 1])
            for j in range(MB):
                sq_reduce(xg[0][:, j, :], pxg[:, j, :], Praw[:, j:j + 1])
            load_group(1)
            squares_group(1)

            # sqrt batch 1: groups 0-1 + partner rows
            nc.scalar.activation(NRM[:, 0:2 * GB], S[:, 0:2 * GB], AF.Sqrt)
            nc.scalar.activation(NRMPT[:], SPT[:], AF.Sqrt)
            nc.vector.tensor_scalar(
                out=RS[:, 0:2 * GB], in0=NRM[:, 0:2 * GB],
                scalar1=1.0 / ZSC, scalar2=None, op0=ALU.mult)
            nc.vector.tensor_scalar(
                out=RSPT[:], in0=NRMPT[:], scalar1=1.0 / ZSC, scalar2=None,
                op0=ALU.mult)
            nc.vector.reciprocal(R16[:, 0:2 * GB], RS[:, 0:2 * GB])
            nc.vector.reciprocal(R16PT[:], RSPT[:])

            # positives / t = praw * (16/ni) * (16/nj) / 128 (exact-ish f32)
            nc.vector.tensor_mul(P2[:], Praw[:], R16[:, 0:MB])
            nc.vector.tensor_mul(P2[:], P2[:], R16PT[:])
            nc.vector.tensor_scalar(
                out=P2[:], in0=P2[:], scalar1=EXPS, scalar2=None, op0=ALU.mult)
            nc.scalar.activation(NOM[:], P2[:], AF.Exp)

            prep_group(0)
            prep_group(1)

            # remaining loads + squares (stream; sqrt batch 2 comes later)
            for g in range(2, NG):
                load_group(g)
                squares_group(g)

            # ---- main loop: 4 column waves x 8 row blocks ----
            def mm_tile(ngi, m):
                ps = pmm.tile([P, COLS], F32, name=f"ps{ngi}_{m}", tag="mm")
                if USE_DR:
                    for kp in range(0, KD, 2):
                        lhsT = ZT[:, kp:kp + 2, m, :]
                        for ns in range(4):
                            b0 = ngi * (COLS // P) + ns * 4
                            rhs = ZT[:, kp:kp + 2, b0:b0 + 4, :]
                            nc.tensor.matmul(
                                ps[:, ns * 512:(ns + 1) * 512], lhsT, rhs,
                                start=(kp == 0), stop=False, perf_mode=DR)
                else:
                    for k in range(KD):
                        lhsT = ZT[:, k, m, :]
                        for ns in range(4):
                            b0 = ngi * (COLS // P) + ns * 4
                            rhs = ZT[:, k, b0:b0 + 4, :]
                            nc.tensor.matmul(
                                ps[:, ns * 512:(ns + 1) * 512], lhsT, rhs,
                                start=(k == 0), stop=False)
                for ns in range(4):
                    c0 = ngi * COLS + ns * 512
                    nc.tensor.matmul(
                        ps[:, ns * 512:(ns + 1) * 512],
                        LTa[:, m * P:(m + 1) * P], LTb[:, c0:c0 + 512],
                        start=False, stop=True)
                e = ebuf.tile([P, COLS], BF16, name=f"e{ngi}_{m}", tag="e")
                nc.scalar.activation(
                    e[:], ps[:], AF.Exp, scale=EXPS,
                    accum_out=ACC[:, m, ngi:ngi + 1])

            for ngi in range(NW):
                for m in range(MB):
                    mm_tile(ngi, m)
                    if ngi == 0 and m == 3:
                        # sqrt batch 2: remaining groups (loads done by now)
                        nc.scalar.activation(
                            NRM[:, 2 * GB:], S[:, 2 * GB:], AF.Sqrt)
                        nc.vector.tensor_scalar(
                            out=RS[:, 2 * GB:], in0=NRM[:, 2 * GB:],
                            scalar1=1.0 / ZSC, scalar2=None, op0=ALU.mult)
                        nc.vector.reciprocal(R16[:, 2 * GB:], RS[:, 2 * GB:])
                    if ngi == 0 and m == 5:
                        prep_group(2)
                        prep_group(3)
                    if ngi == 1 and m == 4:
                        prep_group(4)
                        prep_group(5)
                    if ngi == 2 and m == 4:
                        prep_group(6)
                        prep_group(7)

            # ---- batched epilogue ----
            nc.vector.tensor_reduce(DSUM[:], ACC[:], axis=AX.X, op=ALU.add)
            nc.vector.tensor_add(DEN[:], DSUM[:], NOM[:])
            nc.scalar.activation(LOSS[:], DEN[:], AF.Ln, bias=EPS[:])
            nc.vector.tensor_sub(LOSS2[:], LOSS[:], P2[:])
            nc.vector.tensor_reduce(TOT[:], LOSS2[:], axis=AX.X, op=ALU.add)
            psc = pmm.tile([1, 1], F32, name="psc", tag="mm")
            nc.tensor.matmul(psc[:], TOT[:], ONE[:], start=True, stop=True)
            osb = small.tile([1, 1], F32, name="osb")
            nc.scalar.copy(osb[:], psc[:])
            nc.sync.dma_start(out=out_loss, in_=osb[:])

    nc.compile()
    return nc


_NC_CACHE = None
LAST_RESULTS = None  # test harness can read exec_time_ns / trace from here


def _get_nc():
    global _NC_CACHE
    if _NC_CACHE is None:
        _NC_CACHE = build_program()
    return _NC_CACHE


def kernel(emb_i, emb_j, target):
    emb_i = np.ascontiguousarray(emb_i, dtype=np.float32)
    emb_j = np.ascontiguousarray(emb_j, dtype=np.float32)
    target = np.asarray(target)

    X = np.concatenate([emb_i, emb_j], axis=0).astype(ml_dtypes.bfloat16)
    labels = np.concatenate([target, target]).astype(np.float32)
    labels_bf = labels.astype(ml_dtypes.bfloat16)

    iota_p = np.arange(P, dtype=np.float32).reshape(P, 1)
    ones_p = np.ones((P, 1), dtype=np.float32)

    in_maps = []
    for c in range(NCORES):
        lo = c * MYR
        x_rot = np.ascontiguousarray(np.concatenate([X[lo:], X[:lo]], axis=0))
        lab_rot = np.ascontiguousarray(
            np.concatenate([labels_bf[lo:], labels_bf[:lo]])).reshape(1, N2)
        pt_idx = (np.arange(lo, lo + MYR) + B) % N2
        in_maps.append({
            "x_rot": x_rot,
            "pt_x": np.ascontiguousarray(X[pt_idx]),
            "labels_rot": lab_rot,
            "iota_p": iota_p,
            "ones_p": ones_p,
        })

    nc = _get_nc()
    res = run_bass_kernel_spmd(nc, in_maps, core_ids=list(range(NCORES)))
    global LAST_RESULTS
    LAST_RESULTS = res
    total = 0.0
    for c in range(NCORES):
        total += float(res.results[c]["out_loss"][0, 0])
    return np.float32(total / N2)


# revision 23
# speedup vs baseline: 2.8908x; 1.0549x over previous
"""Trainium2 Bass kernel for nn_ContrastiveLoss (NT-Xent / SimCLR loss).

B=4096, D=512, 100 classes, temperature 0.5.  loss =
  mean_i [ log(denom_i + 1e-7) - pos_i/t ]
where denom_i = sum_{j: label_j != label_i} exp(sim_ij/t) + exp(pos_i/t).

Distribution: rows of the similarity matrix are sharded across 8 cores
(1024 rows each).  Host passes bf16 inputs ROTATED per core so that rows
0..1023 of x_rot are the core's own rows — the core then builds the
transposed, normalized, fp8 representation matrix ZT column-group by
column-group starting with its own rows, so the first matmul wave starts
~15 us in and the PE stays continuously busy (keeps the HAM clock warm).

Compute plan (per core):
  PE   - fp8e4 DoubleRow matmuls: z16 = 16*x/||x|| in fp8, K=512 done as
         2 pair-chunks of (2x128); PSUM gets 256*sim.  Label mask folded
         in as a bf16 one-hot matmul: -64*onehot x 64*onehot accumulates
         -4096 into same-label entries; exp((256*sim-4096)/128) = 0.
  ACT  - ONLY Exp over [128, 2048] PSUM tiles (fused accum_out = row
         denominator) + two batched Sqrt instrs + final Ln: 5 activation
         table loads total instead of 16.
  DVE  - square+reduce pairs for row norms (tensor_tensor_reduce is NOT
         HW-safe: it passes CoreSim but hangs the device), reciprocal,
         z16 scaling, bf16->fp8 convert of transposed tiles, one-hot.
  DMA  - bf16 input loads (half the f32 traffic) + xbar transposes.
ZT is stored k-major [kappa][k_chunk][block][rho] so DoubleRow rhs slices
optimize to legal 3D APs with 512-wide moving operands.
Positives are recomputed exactly from bf16 inputs (praw * 16/ni * 16/nj
/ 128) and re-added to the denominator.
Final reduction: ones-vector matmul -> [1,1]; host sums 8 partials.
"""

import sys

for _p in ("/opt/trn_rl_repo", "/root/.axon_site/_ro/trn_rl_repo"):
    if _p not in sys.path:
        sys.path.append(_p)

import numpy as np
import ml_dtypes

import concourse.bass as bass
import concourse.bacc as bacc
import concourse.mybir as mybir
from concourse import tile
from concourse.bass_utils import run_bass_kernel_spmd

F32 = mybir.dt.float32
BF16 = mybir.dt.bfloat16
FP8 = mybir.dt.float8e4
AF = mybir.ActivationFunctionType
ALU = mybir.AluOpType
AX = mybir.AxisListType
DR = mybir.MatmulPerfMode.DoubleRow

P = 128          # partitions
B = 4096         # batch
D = 512          # embedding dim
N2 = 2 * B       # 8192 rows of sim matrix
NCORES = 8
MYR = N2 // NCORES          # 1024 rows per core
NB = N2 // P                # 64 row blocks total
MB = MYR // P               # 8 row blocks per core
KD = D // P                 # 4 contraction chunks
GB = 8                      # row blocks per load group
NG = 8                      # load groups
COLS = 2048                 # psum tile columns
NW = N2 // COLS             # 4 column waves
ZSC = 16.0                  # fp8 scale: z16 = 16 * z
EXPS = 2.0 / (ZSC * ZSC)    # exp scale: 2/256 = 1/128 (temp 0.5)
MASK_W = 64.0               # -64*64 = -4096 -> exp shift -32
USE_DR = True               # DoubleRow fp8 matmuls (2 k-chunks / instr)


def build_program():
    nc = bacc.Bacc("TRN2", target_bir_lowering=False, debug=False)

    x_rot = nc.dram_tensor("x_rot", [N2, D], BF16, kind="ExternalInput").ap()
    pt_x = nc.dram_tensor("pt_x", [MYR, D], BF16, kind="ExternalInput").ap()
    labels_rot = nc.dram_tensor("labels_rot", [1, N2], BF16, kind="ExternalInput").ap()
    iota_p = nc.dram_tensor("iota_p", [P, 1], F32, kind="ExternalInput").ap()
    ones_p = nc.dram_tensor("ones_p", [P, 1], F32, kind="ExternalInput").ap()
    out_loss = nc.dram_tensor("out_loss", [1, 1], F32, kind="ExternalOutput").ap()

    with tile.TileContext(nc) as tc:
        with (
            tc.tile_pool(name="big", bufs=1) as big,
            tc.tile_pool(name="xin", bufs=1) as xin,
            tc.tile_pool(name="zs", bufs=2) as zs,
            tc.tile_pool(name="ztb", bufs=2) as ztb,
            tc.tile_pool(name="ebuf", bufs=3) as ebuf,
            tc.tile_pool(name="small", bufs=1) as small,
            tc.tile_pool(name="pmm", bufs=2, space=bass.MemorySpace.PSUM) as pmm,
        ):
            # ---- persistent tiles ----
            ZT = big.tile([P, KD, NB, P], FP8, name="ZT")      # [kappa][k][blk][rho]
            LTb = big.tile([P, N2], BF16, name="LTb")          # 64*onehot (rhs)
            LTa = big.tile([P, MYR], BF16, name="LTa")         # -64*onehot (lhsT)

            S = small.tile([P, NB], F32, name="S")             # ||x||^2 per row
            SPT = small.tile([P, MB], F32, name="SPT")
            NRM = small.tile([P, NB], F32, name="NRM")
            NRMPT = small.tile([P, MB], F32, name="NRMPT")
            RS = small.tile([P, NB], F32, name="RS")           # nrm/16
            RSPT = small.tile([P, MB], F32, name="RSPT")
            R16 = small.tile([P, NB], F32, name="R16")         # 16/nrm
            R16PT = small.tile([P, MB], F32, name="R16PT")
            SCR = small.tile([P, D], BF16, name="SCR")         # DVE scratch
            SQ = small.tile([P, D], BF16, name="SQ")           # ACT sq scratch
            Praw = small.tile([P, MB], F32, name="Praw")
            P2 = small.tile([P, MB], F32, name="P2")           # positives / t
            ACC = small.tile([P, MB, NW], F32, name="ACC")
            DSUM = small.tile([P, MB], F32, name="DSUM")
            NOM = small.tile([P, MB], F32, name="NOM")
            DEN = small.tile([P, MB], F32, name="DEN")
            LOSS = small.tile([P, MB], F32, name="LOSS")
            LOSS2 = small.tile([P, MB], F32, name="LOSS2")
            TOT = small.tile([P, 1], F32, name="TOT")
            IOT = small.tile([P, 1], F32, name="IOT")
            ONE = small.tile([P, 1], F32, name="ONE")
            EPS = small.tile([P, 1], F32, name="EPS")
            nc.vector.memset(EPS[:], 1e-7)

            nc.sync.dma_start(out=IOT[:], in_=iota_p)
            nc.sync.dma_start(out=ONE[:], in_=ones_p)

            # ---- input loads (bf16) ----
            xg = []
            for g in range(NG):
                t = xin.tile([P, GB, D], BF16, name=f"xg{g}", tag="xg", bufs=NG)
                xg.append(t)
            pxg = xin.tile([P, MB, D], BF16, name="pxg", tag="px", bufs=1)

            def load_group(g):
                src = x_rot[g * GB * P:(g + 1) * GB * P, :].rearrange(
                    "(b p) d -> p b d", p=P)
                nc.sync.dma_start(out=xg[g][:], in_=src)

            def sq_reduce(in0, in1, acc):
                nc.vector.tensor_mul(SCR[:], in0, in1)
                nc.vector.tensor_reduce(acc, SCR[:], axis=AX.X, op=ALU.add)

            def squares_group(g):
                for j in range(GB):
                    b = g * GB + j
                    sq_reduce(xg[g][:, j, :], xg[g][:, j, :], S[:, b:b + 1])

            def prep_group(g):
                # z16 = x * (16/||x||)  (bf16) -> transpose -> fp8 convert
                zg = zs.tile([P, GB, D], BF16, name=f"zg{g}", tag="zg")
                for j in range(GB):
                    b = g * GB + j
                    nc.vector.tensor_scalar(
                        out=zg[:, j, :], in0=xg[g][:, j, :],
                        scalar1=R16[:, b:b + 1], scalar2=None, op0=ALU.mult)
                zt = ztb.tile([P, GB, KD, P], BF16, name=f"zt{g}", tag="zt")
                nc.scalar.dma_start_transpose(out=zt[:], in_=zg[:])
                for k in range(KD):
                    nc.vector.tensor_scalar(
                        out=ZT[:, k, g * GB:(g + 1) * GB, :],
                        in0=zt[:, :, k, :],
                        scalar1=1.0, scalar2=None, op0=ALU.mult)

            # sync queue: g0, g1 first (gate wave 0), then labels, pt, rest
            load_group(0)
            load_group(1)
            nc.sync.dma_start(out=LTb[:], in_=labels_rot.partition_broadcast(P))
            nc.sync.dma_start(out=pxg[:], in_=pt_x.rearrange("(b p) d -> p b d", p=P))
            for g in range(2, NG):
                load_group(g)

            # critical path to wave 0: squares g0,g1 -> sqrt1 -> prep g0,g1
            squares_group(0)
            squares_group(1)
            nc.scalar.activation(NRM[:, 0:2 * GB], S[:, 0:2 * GB], AF.Sqrt)
            nc.vector.tensor_scalar(
                out=RS[:, 0:2 * GB], in0=NRM[:, 0:2 * GB],
                scalar1=1.0 / ZSC, scalar2=None, op0=ALU.mult)
            nc.vector.reciprocal(R16[:, 0:2 * GB], RS[:, 0:2 * GB])
            prep_group(0)
            prep_group(1)

            # off the critical path: one-hot builds (DVE), g2-7 squares (ACT
            # Square + fused accum, keeps the DVE queue free for preps),
            # partner squares + praw (DVE)
            nc.vector.tensor_scalar(
                out=LTa[:], in0=LTb[:, 0:MYR], scalar1=IOT[:], scalar2=-MASK_W,
                op0=ALU.is_equal, op1=ALU.mult,
            )
            nc.vector.tensor_scalar(
                out=LTb[:], in0=LTb[:], scalar1=IOT[:], scalar2=MASK_W,
                op0=ALU.is_equal, op1=ALU.mult,
            )
            for g in range(2, NG):
                for j in range(GB):
                    b = g * GB + j
                    nc.scalar.activation(
                        SQ[:], xg[g][:, j, :], AF.Square,
                        accum_out=S[:, b:b + 1])
            for j in range(MB):
                nc.scalar.activation(
                    SQ[:], pxg[:, j, :], AF.Square, accum_out=SPT[:, j:j + 1])
            for j in range(MB):
                sq_reduce(xg[0][:, j, :], pxg[:, j, :], Praw[:, j:j + 1])

            # ---- main loop: 4 column waves x 8 row blocks ----
            def mm_tile(ngi, m):
                ps = pmm.tile([P, COLS], F32, name=f"ps{ngi}_{m}", tag="mm")
                if USE_DR:
                    for kp in range(0, KD, 2):
                        lhsT = ZT[:, kp:kp + 2, m, :]
                        for ns in range(4):
                            b0 = ngi * (COLS // P) + ns * 4
                            rhs = ZT[:, kp:kp + 2, b0:b0 + 4, :]
                            nc.tensor.matmul(
                                ps[:, ns * 512:(ns + 1) * 512], lhsT, rhs,
                                start=(kp == 0), stop=False, perf_mode=DR)
                else:
                    for k in range(KD):
                        lhsT = ZT[:, k, m, :]
                        for ns in range(4):
                            b0 = ngi * (COLS // P) + ns * 4
                            rhs = ZT[:, k, b0:b0 + 4, :]
                            nc.tensor.matmul(
                                ps[:, ns * 512:(ns + 1) * 512], lhsT, rhs,
                                start=(k == 0), stop=False)
                for ns in range(4):
                    c0 = ngi * COLS + ns * 512
                    nc.tensor.matmul(
                        ps[:, ns * 512:(ns + 1) * 512],
                        LTa[:, m * P:(m + 1) * P], LTb[:, c0:c0 + 512],
                        start=False, stop=True)
                e = ebuf.tile([P, COLS], BF16, name=f"e{ngi}_{m}", tag="e")
                nc.scalar.activation(
                    e[:], ps[:], AF.Exp, scale=EXPS,
                    accum_out=ACC[:, m, ngi:ngi + 1])

            for ngi in range(NW):
                for m in range(MB):
                    mm_tile(ngi, m)
                    if ngi == 0 and m == 3:
                        # sqrt batch 2: remaining groups + partner rows
                        nc.scalar.activation(
                            NRM[:, 2 * GB:], S[:, 2 * GB:], AF.Sqrt)
                        nc.scalar.activation(NRMPT[:], SPT[:], AF.Sqrt)
                        nc.vector.tensor_scalar(
                            out=RS[:, 2 * GB:], in0=NRM[:, 2 * GB:],
                            scalar1=1.0 / ZSC, scalar2=None, op0=ALU.mult)
                        nc.vector.reciprocal(R16[:, 2 * GB:], RS[:, 2 * GB:])
                    if ngi == 0 and m == 4:
                        # positives / t = praw * (16/ni) * (16/nj) / 128
                        nc.vector.tensor_scalar(
                            out=RSPT[:], in0=NRMPT[:], scalar1=1.0 / ZSC,
                            scalar2=None, op0=ALU.mult)
                        nc.vector.reciprocal(R16PT[:], RSPT[:])
                        nc.vector.tensor_mul(P2[:], Praw[:], R16[:, 0:MB])
                        nc.vector.tensor_mul(P2[:], P2[:], R16PT[:])
                        nc.vector.tensor_scalar(
                            out=P2[:], in0=P2[:], scalar1=EXPS, scalar2=None,
                            op0=ALU.mult)
                    if ngi == 0 and m == 6:
                        nc.scalar.activation(NOM[:], P2[:], AF.Exp)
                    if ngi == 0 and m == 5:
                        prep_group(2)
                        prep_group(3)
                    if ngi == 1 and m == 4:
                        prep_group(4)
                        prep_group(5)
                    if ngi == 2 and m == 4:
                        prep_group(6)
                        prep_group(7)

            # ---- batched epilogue ----
            nc.vector.tensor_reduce(DSUM[:], ACC[:], axis=AX.X, op=ALU.add)
            nc.vector.tensor_add(DEN[:], DSUM[:], NOM[:])
            nc.scalar.activation(LOSS[:], DEN[:], AF.Ln, bias=EPS[:])
            nc.vector.tensor_sub(LOSS2[:], LOSS[:], P2[:])
            nc.vector.tensor_reduce(TOT[:], LOSS2[:], axis=AX.X, op=ALU.add)
            psc = pmm.tile([1, 1], F32, name="psc", tag="mm")
            nc.tensor.matmul(psc[:], TOT[:], ONE[:], start=True, stop=True)
            osb = small.tile([1, 1], F32, name="osb")
            nc.scalar.copy(osb[:], psc[:])
            nc.sync.dma_start(out=out_loss, in_=osb[:])

    nc.compile()
    return nc


_NC_CACHE = None
LAST_RESULTS = None  # test harness can read exec_time_ns / trace from here


def _get_nc():
    global _NC_CACHE
    if _NC_CACHE is None:
        _NC_CACHE = build_program()
    return _NC_CACHE


def kernel(emb_i, emb_j, target):
    emb_i = np.ascontiguousarray(emb_i, dtype=np.float32)
    emb_j = np.ascontiguousarray(emb_j, dtype=np.float32)
    target = np.asarray(target)

    X = np.concatenate([emb_i, emb_j], axis=0).astype(ml_dtypes.bfloat16)
    labels = np.concatenate([target, target]).astype(np.float32)
    labels_bf = labels.astype(ml_dtypes.bfloat16)

    iota_p = np.arange(P, dtype=np.float32).reshape(P, 1)
    ones_p = np.ones((P, 1), dtype=np.float32)

    in_maps = []
    for c in range(NCORES):
        lo = c * MYR
        x_rot = np.ascontiguousarray(np.concatenate([X[lo:], X[:lo]], axis=0))
        lab_rot = np.ascontiguousarray(
            np.concatenate([labels_bf[lo:], labels_bf[:lo]])).reshape(1, N2)
        pt_idx = (np.arange(lo, lo + MYR) + B) % N2
        in_maps.append({
            "x_rot": x_rot,
            "pt_x": np.ascontiguousarray(X[pt_idx]),
            "labels_rot": lab_rot,
            "iota_p": iota_p,
            "ones_p": ones_p,
        })

    nc = _get_nc()
    res = run_bass_kernel_spmd(nc, in_maps, core_ids=list(range(NCORES)))
    global LAST_RESULTS
    LAST_RESULTS = res
    total = 0.0
    for c in range(NCORES):
        total += float(res.results[c]["out_loss"][0, 0])
    return np.float32(total / N2)
